# revision 52
# baseline (speedup 1.0000x reference)
"""TRN2 Bass kernel: out = (A@x)/deg @ W.T + x @ B.T  (graph conv, set-semantics A).

Self-contained. Shards destination rows across 8 NeuronCores (row-parallel
SpMM). Host does integer-only edge prep (dedup/balancing/window-scheduling/
padding) plus the x-row gather relayout and dtype casts; all FLOPs run on
device: one-hot segment-sum matmuls, degree normalization, the W projection.

Per core (2048 destination rows, ~64.5k deduped edges):
  - Destinations are load-balanced across blocks (LPT by degree) so K=16
    chunks of 128 edge slots suffice (98.4% fill). Host un-permutes rows.
  - Chunk j of a block accepts only edges whose dst position falls in a FIXED
    window (chunk 0: full 64-wide; others: 32-wide sliding) — SPMD-static
    matmul APs.
  - Mixed precision: chunk positions have a fixed fp16/fp8(e4m3) class
    pattern. Each destination d may have at most (theta*deg_d)^2 of its edges
    in fp8 chunks, which bounds every destination's quantization error;
    high-degree destinations go mostly fp8. fp16-class edges must be packed
    into fp16 chunks (fp8-class edges may spill into fp16 chunks). Gathered
    x rows are stored per-chunk-dtype in one contiguous byte blob per block
    -> one DMA per block, ~11MB/core instead of 16.8MB (DMA is the
    bottleneck: HBM-ceiling bound).
  - one-hot scatter matrices: three broadcast is_equal ops per BPV blocks on
    the vector engine (chunk0 / fp16-windowed / fp8-windowed) against
    on-device iotas.
  - yt PSUM->SBUF copies on the scalar engine; deg-normalization is one
    vector multiply against a partition-broadcast 1/deg row (fast approx
    reciprocal). Output fp16 [F, 2048]; host un-transposes/per-mutes and
    casts to fp32.
"""

import os
import numpy as np
from contextlib import ExitStack

import concourse.bass as bass
import concourse.bacc as bacc
import concourse.mybir as mybir
import concourse.tile as tile
from concourse.bass_utils import run_bass_kernel_spmd

F = 128
BLK = 128
IBW = 64        # destination-block width (dst columns per PSUM tile)
WIN = 32        # chunk window width (non-first chunks)
N_CORES = 8
BPV = 4         # blocks per one-hot vector op / blocks per gin DMA
PROJ_W = 512    # projection pass width (dst cols per stationary-W matmul)
THETA = 0.15    # per-dst fp8 budget: n_fp8(d) <= (THETA*deg)^2
FP16_T = (1, 5, 9, 13)  # windowed fp16 chunk positions (original t)


def _lo_of(t):
    return int(np.clip(4 * (t - 4), 0, IBW - WIN))


def _chunk_config(K):
    """Returns list of (lo, width, is8) in permuted chunk order:
    [chunk0 fp16 full-width] + [fp16 windowed] + [fp8 windowed]."""
    assert K >= 13
    f16 = [t for t in FP16_T if t < K]
    f8 = [t for t in range(1, K) if t not in f16]
    order = [0] + f16 + f8
    cfg = []
    for j, t in enumerate(order):
        if t == 0:
            cfg.append((0, IBW, False))
        else:
            cfg.append((_lo_of(t), WIN, j > len(f16)))
    return cfg


def _host_prep(x, edge_index, n_cores=N_CORES):
    N = x.shape[0]
    src = edge_index[0].astype(np.int64)
    dst = edge_index[1].astype(np.int64)
    keys = np.unique(dst * N + src)  # set semantics + sort by (dst, src)
    dst_u = (keys // N).astype(np.int32)
    src_u = (keys % N).astype(np.int32)
    deg = np.bincount(dst_u, minlength=N).astype(np.int32)

    n_gblk = N // IBW
    n_blk = n_gblk // n_cores

    # Balance destination-block loads (LPT by degree) -> minimal K.
    import heapq
    heap = [(0, g) for g in range(n_gblk)]
    heapq.heapify(heap)
    slots = np.zeros(n_gblk, np.int32)
    newidx = np.empty(N, np.int32)
    # Within a block, dsts arrive in degree-desc order; interleave their
    # positions so low-degree (fp16-heavy) dsts spread across the whole
    # position range (the fp16 chunk windows each cover only part of it).
    pos64 = np.concatenate([np.arange(0, IBW, 2), np.arange(1, IBW, 2)])
    for d in np.argsort(-deg, kind="stable"):
        while True:
            load, g = heapq.heappop(heap)
            if slots[g] < IBW:
                break
        newidx[d] = g * IBW + int(pos64[slots[g]])
        slots[g] += 1
        heapq.heappush(heap, (load + int(deg[d]), g))

    perm = np.empty(N, np.int64)          # perm[global_pos] = orig dst id
    perm[newidx] = np.arange(N)
    degp = np.zeros(N, np.int32)
    degp[newidx] = deg                    # degree per global position

    ekey = newidx[dst_u]
    order_e = np.lexsort((src_u, ekey))
    pos_s = ekey[order_e]
    src_s = src_u[order_e]
    counts = np.bincount(pos_s // IBW, minlength=n_gblk)
    # +1 slack chunk: the two-class window packing needs headroom beyond the
    # exact-fit block load (max block is at ~100% of K*BLK slots).
    K = max(int(np.ceil(counts.max() / BLK)), 13) + 1
    cfg = _chunk_config(K)

    # per-edge class: for each dst position, the last n8 of its edges are
    # fp8-class; n8 = min(deg, floor((theta*deg)^2))
    deg_pos = degp  # by global position
    n8_of = np.minimum(deg_pos, ((THETA * deg_pos) ** 2).astype(np.int64))
    # rank of each edge within its dst run
    rank = np.arange(len(pos_s)) - np.repeat(
        np.concatenate(([0], np.cumsum(np.bincount(pos_s, minlength=N))[:-1])),
        np.bincount(pos_s, minlength=N))
    is8_edge = rank >= (deg_pos[pos_s] - n8_of[pos_s])

    bptr = np.zeros(n_gblk + 1, np.int64)
    np.cumsum(counts, out=bptr[1:])

    # class-aware window packing (permuted chunk order)
    nJ = len(cfg)
    f16_idx = [j for j, (_, _, e) in enumerate(cfg) if not e]
    f8_idx = [j for j, (_, _, e) in enumerate(cfg) if e]
    # preference orders: windowed chunks ascending lo, chunk0 last for fp16
    pref16 = sorted(f16_idx[1:], key=lambda j: cfg[j][0]) + [0]
    pref8 = sorted(f8_idx, key=lambda j: cfg[j][0]) + pref16

    src_slot = np.zeros((n_cores, n_blk, nJ, BLK), np.int32)
    drw = np.full((n_cores, n_blk, nJ, BLK), -1.0, np.float16)
    for g in range(n_gblk):
        c, b = divmod(g, n_blk)
        s, e = int(bptr[g]), int(bptr[g + 1])
        d_rel = pos_s[s:e] - g * IBW     # ascending
        srcs = src_s[s:e]
        e8 = is8_edge[s:e]
        fill = [0] * nJ
        for i in range(e - s):
            d = int(d_rel[i])
            pref = pref8 if e8[i] else pref16
            for j in pref:
                lo, w, _ = cfg[j]
                if lo <= d < lo + w and fill[j] < BLK:
                    break
            else:
                raise AssertionError(
                    f"packing infeasible block {g} dst {d} is8={e8[i]}")
            src_slot[c, b, j, fill[j]] = srcs[i]
            drw[c, b, j, fill[j]] = np.float16(d - cfg[j][0])
            fill[j] += 1

    # gin byte blob: per block row = concat over chunks of that chunk's
    # gathered x row in the chunk dtype.
    x16 = x.astype(np.float16)
    x8 = x.astype(mybir.dt.np(mybir.dt.float8e4))
    parts = []
    for j, (_, _, is8) in enumerate(cfg):
        sl = src_slot[:, :, j, :]                      # [c, b, p]
        q = (x8 if is8 else x16)[sl]                   # [c, b, p, F]
        parts.append(np.ascontiguousarray(q).view(np.uint8).reshape(
            n_cores, n_blk, BLK, -1))
    ginb = np.concatenate(parts, axis=3)               # [c, b, p, ROWB]
    rowb = ginb.shape[3]
    # expose the byte blob as fp16 I/O (same bytes; avoids uint8 externals)
    gin = np.ascontiguousarray(
        ginb.reshape(n_cores, n_blk * BLK, rowb)).view(np.float16)

    dr = np.ascontiguousarray(
        drw.transpose(0, 3, 1, 2).reshape(n_cores, BLK, n_blk * nJ))
    degc = np.ascontiguousarray(degp.reshape(n_cores, 1, n_blk * IBW))
    return gin, dr, degc, K, n_blk, perm


def _build_program(N, n_blk, K, rowb):
    nc = bacc.Bacc("TRN2", target_bir_lowering=False, num_devices=N_CORES)
    ND = n_blk * IBW  # destinations per core (2048)
    cfg = _chunk_config(K)
    nJ = len(cfg)
    n16 = sum(1 for _, _, e in cfg if not e) - 1   # windowed fp16 chunks
    n8 = nJ - 1 - n16
    # byte offsets of each chunk within a block row
    offs = np.cumsum([0] + [F * (1 if e else 2) for _, _, e in cfg]).tolist()
    gin = nc.dram_tensor("gin", [n_blk * BLK, rowb // 2], mybir.dt.float16, kind="ExternalInput")
    drd = nc.dram_tensor("dr", [BLK, n_blk * nJ], mybir.dt.float16, kind="ExternalInput")
    degd = nc.dram_tensor("deg", [1, ND], mybir.dt.int32, kind="ExternalInput")
    wtd = nc.dram_tensor("wt", [F, F], mybir.dt.float16, kind="ExternalInput")
    out = nc.dram_tensor("out", [F, ND], mybir.dt.float16, kind="ExternalOutput")

    n_it = n_blk // BPV
    n_pg = ND // PROJ_W
    it_per_pg = n_it // n_pg

    with tile.TileContext(nc) as tc, ExitStack() as ctx:
        const = ctx.enter_context(tc.tile_pool(name="const", bufs=1))
        gpool = ctx.enter_context(tc.tile_pool(name="g", bufs=4))
        spool = ctx.enter_context(tc.tile_pool(name="s", bufs=3))
        opool = ctx.enter_context(tc.tile_pool(name="o", bufs=2))
        psum = ctx.enter_context(tc.tile_pool(name="ps", bufs=3, space="PSUM"))
        psum2 = ctx.enter_context(tc.tile_pool(name="ps2", bufs=2, space="PSUM"))

        dr_t = const.tile([BLK, n_blk, nJ], mybir.dt.float16)
        nc.sync.dma_start(dr_t[:], drd[:])

        iota64 = const.tile([BLK, BPV, IBW], mybir.dt.float16)
        nc.gpsimd.iota(iota64[:], pattern=[[0, BPV], [1, IBW]], base=0,
                       channel_multiplier=0,
                       allow_small_or_imprecise_dtypes=True)
        iota16 = const.tile([BLK, BPV, n16, WIN], mybir.dt.float16)
        nc.gpsimd.iota(iota16[:], pattern=[[0, BPV], [0, n16], [1, WIN]],
                       base=0, channel_multiplier=0,
                       allow_small_or_imprecise_dtypes=True)
        iota8 = const.tile([BLK, BPV, n8, WIN], mybir.dt.float16)
        nc.gpsimd.iota(iota8[:], pattern=[[0, BPV], [0, n8], [1, WIN]],
                       base=0, channel_multiplier=0,
                       allow_small_or_imprecise_dtypes=True)

        wt_t = const.tile([F, F], mybir.dt.float16)
        deg_i = const.tile([1, ND], mybir.dt.int32)
        deg_f = const.tile([1, ND], mybir.dt.float32)
        rdeg1 = const.tile([1, ND], mybir.dt.float32)
        rdeg_b = const.tile([BLK, ND], mybir.dt.float32)
        yt_all = const.tile([BLK, ND], mybir.dt.float16)

        def emit_consts():
            nc.sync.dma_start(wt_t[:], wtd[:])
            nc.sync.dma_start(deg_i[:], degd[:])
            nc.vector.tensor_copy(deg_f[:], deg_i[:])
            nc.vector.reciprocal_approx_fast(rdeg1[:], deg_f[:])
            nc.gpsimd.partition_broadcast(rdeg_b[:], rdeg1[:])

        def emit_proj(pg):
            o_ps = psum2.tile([F, PROJ_W], mybir.dt.float32, tag="op")
            nc.tensor.matmul(
                o_ps[:], lhsT=wt_t[:],
                rhs=yt_all[:, pg * PROJ_W:(pg + 1) * PROJ_W],
                start=True, stop=True,
            )
            o_sb = opool.tile([F, PROJ_W], mybir.dt.float16, tag="ob")
            nc.vector.tensor_tensor(
                out=o_sb[:], in0=o_ps[:],
                in1=rdeg_b[:, pg * PROJ_W:(pg + 1) * PROJ_W],
                op=mybir.AluOpType.mult,
            )
            nc.scalar.dma_start(out[:, pg * PROJ_W:(pg + 1) * PROJ_W], o_sb[:])

        RW = rowb // 2
        for i in range(n_it):
            # per-block DMAs into one tile: subtile deps let block j's
            # matmuls start as soon as its slice lands (finer pipelining
            # than one BPV-block transfer).
            g2 = gpool.tile([BLK, BPV, RW], mybir.dt.float16, tag="g")
            for j in range(BPV):
                b = i * BPV + j
                nc.sync.dma_start(g2[:, j, :], gin[b * BLK:(b + 1) * BLK, :])
            g_ts = [g2[:, j, :] for j in range(BPV)]
            if i == 1:
                emit_consts()

            s0 = spool.tile([BLK, BPV, IBW], mybir.dt.float16, tag="s0")
            nc.vector.tensor_tensor(
                out=s0[:],
                in0=iota64[:],
                in1=dr_t[:, i * BPV:(i + 1) * BPV, 0]
                .unsqueeze(2).broadcast_to([BLK, BPV, IBW]),
                op=mybir.AluOpType.is_equal,
            )
            s16 = spool.tile([BLK, BPV, n16, WIN], mybir.dt.float16, tag="s16")
            nc.vector.tensor_tensor(
                out=s16[:],
                in0=iota16[:],
                in1=dr_t[:, i * BPV:(i + 1) * BPV, 1:1 + n16]
                .unsqueeze(3).broadcast_to([BLK, BPV, n16, WIN]),
                op=mybir.AluOpType.is_equal,
            )
            s8 = spool.tile([BLK, BPV, n8, WIN], mybir.dt.float8e4, tag="s8")
            nc.vector.tensor_tensor(
                out=s8[:],
                in0=iota8[:],
                in1=dr_t[:, i * BPV:(i + 1) * BPV, 1 + n16:]
                .unsqueeze(3).broadcast_to([BLK, BPV, n8, WIN]),
                op=mybir.AluOpType.is_equal,
            )

            for j in range(BPV):
                b = i * BPV + j
                yt_ps = psum.tile([F, IBW], mybir.dt.float32, tag="yt")
                g_t = g_ts[j]
                nc.tensor.matmul(
                    yt_ps[:],
                    lhsT=g_t[:, offs[0] // 2:offs[1] // 2],
                    rhs=s0[:, j, :],
                    start=True, stop=False,
                    skip_group_check=True,
                )
                for q in range(1, nJ):
                    lo, w, is8 = cfg[q]
                    lhsT = g_t[:, offs[q] // 2:offs[q + 1] // 2]
                    if is8:
                        lhsT = lhsT.bitcast(mybir.dt.float8e4)
                    rhs = (s8[:, j, q - 1 - n16, :] if is8
                           else s16[:, j, q - 1, :])
                    nc.tensor.matmul(
                        yt_ps[:, lo:lo + w],
                        lhsT=lhsT, rhs=rhs,
                        start=False, stop=(q == nJ - 1),
                        skip_group_check=True,
                    )
                nc.scalar.activation(
                    yt_all[:, b * IBW:(b + 1) * IBW], yt_ps[:],
                    mybir.ActivationFunctionType.Copy,
                )

            if (i + 1) % it_per_pg == 0:
                emit_proj((i + 1) // it_per_pg - 1)

    nc.compile()
    return nc


_PROGRAM_CACHE = {}


def kernel(x, edge_index, W, B, profile_dir=None):
    x = np.ascontiguousarray(np.asarray(x), dtype=np.float32)
    edge_index = np.asarray(edge_index)
    W = np.asarray(W, dtype=np.float32)
    B = np.asarray(B, dtype=np.float32)
    N = x.shape[0]

    gin, dr, degc, K, n_blk, perm = _host_prep(x, edge_index)

    rowb = gin.shape[2] * 2  # bytes per block row (gin is fp16-typed bytes)
    ck = (N, n_blk, K, rowb)
    if ck not in _PROGRAM_CACHE:
        _PROGRAM_CACHE[ck] = _build_program(N, n_blk, K, rowb)
    nc = _PROGRAM_CACHE[ck]

    wt_np = np.ascontiguousarray(W.T.astype(np.float16))
    in_maps = [{
        "gin": gin[c],
        "dr": np.ascontiguousarray(dr[c]),
        "deg": np.ascontiguousarray(degc[c]),
        "wt": wt_np,
    } for c in range(N_CORES)]

    if profile_dir is not None:
        from trn_agent_boot.trn_boot import _ntff_profile_via_ctypes
        hook = _ntff_profile_via_ctypes("/opt/axon/libaxon_pjrt.so")
        os.makedirs(profile_dir, exist_ok=True)
        with hook(profile_dir, list(range(N_CORES))):
            res = run_bass_kernel_spmd(nc, in_maps, core_ids=list(range(N_CORES)))
    else:
        res = run_bass_kernel_spmd(nc, in_maps, core_ids=list(range(N_CORES)))

    rows = np.concatenate(
        [r["out"].T.astype(np.float32) for r in res.results], axis=0)
    out = np.empty_like(rows)
    out[perm] = rows  # un-permute balanced layout back to original dst ids

    if np.any(B):
        # B is zeros for this problem's inputs; exact fallback for generality.
        out = out + x @ B.T
    return out


# revision 53
# speedup vs baseline: 1.1287x; 1.1287x over previous
"""TRN2 Bass kernel: out = (A@x)/deg @ W.T + x @ B.T  (graph conv, set-semantics A).

Self-contained. Shards destination rows across 8 NeuronCores (row-parallel
SpMM). Host does integer-only edge prep (dedup/balancing/window-scheduling/
padding) plus the x-row gather relayout and dtype casts; all FLOPs run on
device: one-hot segment-sum matmuls, degree normalization, the W projection.

Per core (2048 destination rows, ~64.5k deduped edges):
  - Destinations are load-balanced across blocks (LPT by degree) so K=16
    chunks of 128 edge slots suffice (98.4% fill). Host un-permutes rows.
  - Chunk j of a block accepts only edges whose dst position falls in a FIXED
    window (chunk 0: full 64-wide; others: 32-wide sliding) — SPMD-static
    matmul APs.
  - Mixed precision: chunk positions have a fixed fp16/fp8(e4m3) class
    pattern. Each destination d may have at most (theta*deg_d)^2 of its edges
    in fp8 chunks, which bounds every destination's quantization error;
    high-degree destinations go mostly fp8. fp16-class edges must be packed
    into fp16 chunks (fp8-class edges may spill into fp16 chunks). Gathered
    x rows are stored per-chunk-dtype in one contiguous byte blob per block
    -> one DMA per block, ~11MB/core instead of 16.8MB (DMA is the
    bottleneck: HBM-ceiling bound).
  - one-hot scatter matrices: three broadcast is_equal ops per BPV blocks on
    the vector engine (chunk0 / fp16-windowed / fp8-windowed) against
    on-device iotas.
  - yt PSUM->SBUF copies on the scalar engine; deg-normalization is one
    vector multiply against a partition-broadcast 1/deg row (fast approx
    reciprocal). Output fp16 [F, 2048]; host un-transposes/per-mutes and
    casts to fp32.
"""

import os
import numpy as np
from contextlib import ExitStack

import concourse.bass as bass
import concourse.bacc as bacc
import concourse.mybir as mybir
import concourse.tile as tile
from concourse.bass_utils import run_bass_kernel_spmd

F = 128
BLK = 128
IBW = 64        # destination-block width (dst columns per PSUM tile)
WIN = 32        # chunk window width (non-first chunks)
N_CORES = 8
BPV = 4         # blocks per one-hot vector op / blocks per gin DMA
PROJ_W = 512    # projection pass width (dst cols per stationary-W matmul)
THETA = 0.15    # per-dst fp8 budget: n_fp8(d) <= (THETA*deg)^2
FP16_T = (1, 5, 9, 13)  # windowed fp16 chunk positions (original t)


def _lo_of(t):
    return int(np.clip(4 * (t - 4), 0, IBW - WIN))


def _chunk_config(K):
    """Returns list of (lo, width, is8) in permuted chunk order:
    [chunk0 fp16 full-width] + [fp16 windowed] + [fp8 windowed]."""
    assert K >= 13
    f16 = [t for t in FP16_T if t < K]
    f8 = [t for t in range(1, K) if t not in f16]
    order = [0] + f16 + f8
    cfg = []
    for j, t in enumerate(order):
        if t == 0:
            cfg.append((0, IBW, False))
        else:
            cfg.append((_lo_of(t), WIN, j > len(f16)))
    return cfg


def _host_prep(x, edge_index, n_cores=N_CORES):
    N = x.shape[0]
    src = edge_index[0].astype(np.int64)
    dst = edge_index[1].astype(np.int64)
    keys = np.unique(dst * N + src)  # set semantics + sort by (dst, src)
    dst_u = (keys // N).astype(np.int32)
    src_u = (keys % N).astype(np.int32)
    deg = np.bincount(dst_u, minlength=N).astype(np.int32)

    n_gblk = N // IBW
    n_blk = n_gblk // n_cores

    # Balance destination-block loads (LPT by degree) -> minimal K.
    import heapq
    heap = [(0, g) for g in range(n_gblk)]
    heapq.heapify(heap)
    slots = np.zeros(n_gblk, np.int32)
    newidx = np.empty(N, np.int32)
    # Within a block, dsts arrive in degree-desc order; interleave their
    # positions so low-degree (fp16-heavy) dsts spread across the whole
    # position range (the fp16 chunk windows each cover only part of it).
    pos64 = np.concatenate([np.arange(0, IBW, 2), np.arange(1, IBW, 2)])
    for d in np.argsort(-deg, kind="stable"):
        while True:
            load, g = heapq.heappop(heap)
            if slots[g] < IBW:
                break
        newidx[d] = g * IBW + int(pos64[slots[g]])
        slots[g] += 1
        heapq.heappush(heap, (load + int(deg[d]), g))

    perm = np.empty(N, np.int64)          # perm[global_pos] = orig dst id
    perm[newidx] = np.arange(N)
    degp = np.zeros(N, np.int32)
    degp[newidx] = deg                    # degree per global position

    ekey = newidx[dst_u]
    order_e = np.lexsort((src_u, ekey))
    pos_s = ekey[order_e]
    src_s = src_u[order_e]
    counts = np.bincount(pos_s // IBW, minlength=n_gblk)
    # +1 slack chunk: the two-class window packing needs headroom beyond the
    # exact-fit block load (max block is at ~100% of K*BLK slots).
    K = max(int(np.ceil(counts.max() / BLK)), 13) + 1
    cfg = _chunk_config(K)

    # per-edge class: for each dst position, the last n8 of its edges are
    # fp8-class; n8 = min(deg, floor((theta*deg)^2))
    deg_pos = degp  # by global position
    n8_of = np.minimum(deg_pos, ((THETA * deg_pos) ** 2).astype(np.int64))
    # rank of each edge within its dst run
    rank = np.arange(len(pos_s)) - np.repeat(
        np.concatenate(([0], np.cumsum(np.bincount(pos_s, minlength=N))[:-1])),
        np.bincount(pos_s, minlength=N))
    is8_edge = rank >= (deg_pos[pos_s] - n8_of[pos_s])

    bptr = np.zeros(n_gblk + 1, np.int64)
    np.cumsum(counts, out=bptr[1:])

    # class-aware window packing (permuted chunk order)
    nJ = len(cfg)
    f16_idx = [j for j, (_, _, e) in enumerate(cfg) if not e]
    f8_idx = [j for j, (_, _, e) in enumerate(cfg) if e]
    # preference orders: windowed chunks ascending lo, chunk0 last for fp16
    pref16 = sorted(f16_idx[1:], key=lambda j: cfg[j][0]) + [0]
    pref8 = sorted(f8_idx, key=lambda j: cfg[j][0]) + pref16

    src_slot = np.zeros((n_cores, n_blk, nJ, BLK), np.int32)
    drw = np.full((n_cores, n_blk, nJ, BLK), -1.0, np.float16)
    for g in range(n_gblk):
        c, b = divmod(g, n_blk)
        s, e = int(bptr[g]), int(bptr[g + 1])
        d_rel = pos_s[s:e] - g * IBW     # ascending
        srcs = src_s[s:e]
        e8 = is8_edge[s:e]
        fill = [0] * nJ
        for i in range(e - s):
            d = int(d_rel[i])
            pref = pref8 if e8[i] else pref16
            for j in pref:
                lo, w, _ = cfg[j]
                if lo <= d < lo + w and fill[j] < BLK:
                    break
            else:
                raise AssertionError(
                    f"packing infeasible block {g} dst {d} is8={e8[i]}")
            src_slot[c, b, j, fill[j]] = srcs[i]
            drw[c, b, j, fill[j]] = np.float16(d - cfg[j][0])
            fill[j] += 1

    # gin byte blob: per block row = concat over chunks of that chunk's
    # gathered x row in the chunk dtype.
    x16 = x.astype(np.float16)
    x8 = x.astype(mybir.dt.np(mybir.dt.float8e4))
    parts = []
    for j, (_, _, is8) in enumerate(cfg):
        sl = src_slot[:, :, j, :]                      # [c, b, p]
        q = (x8 if is8 else x16)[sl]                   # [c, b, p, F]
        parts.append(np.ascontiguousarray(q).view(np.uint8).reshape(
            n_cores, n_blk, BLK, -1))
    ginb = np.concatenate(parts, axis=3)               # [c, b, p, ROWB]
    rowb = ginb.shape[3]
    # expose the byte blob as fp16 I/O (same bytes; avoids uint8 externals)
    gin = np.ascontiguousarray(
        ginb.reshape(n_cores, n_blk * BLK, rowb)).view(np.float16)

    dr = np.ascontiguousarray(
        drw.transpose(0, 3, 1, 2).reshape(n_cores, BLK, n_blk * nJ))
    degc = np.ascontiguousarray(degp.reshape(n_cores, 1, n_blk * IBW))
    return gin, dr, degc, K, n_blk, perm


def _build_program(N, n_blk, K, rowb):
    nc = bacc.Bacc("TRN2", target_bir_lowering=False, num_devices=N_CORES)
    ND = n_blk * IBW  # destinations per core (2048)
    cfg = _chunk_config(K)
    nJ = len(cfg)
    n16 = sum(1 for _, _, e in cfg if not e) - 1   # windowed fp16 chunks
    n8 = nJ - 1 - n16
    # byte offsets of each chunk within a block row
    offs = np.cumsum([0] + [F * (1 if e else 2) for _, _, e in cfg]).tolist()
    gin = nc.dram_tensor("gin", [n_blk * BLK, rowb // 2], mybir.dt.float16, kind="ExternalInput")
    drd = nc.dram_tensor("dr", [BLK, n_blk * nJ], mybir.dt.float16, kind="ExternalInput")
    degd = nc.dram_tensor("deg", [1, ND], mybir.dt.int32, kind="ExternalInput")
    wtd = nc.dram_tensor("wt", [F, F], mybir.dt.float16, kind="ExternalInput")
    out = nc.dram_tensor("out", [F, ND], mybir.dt.float16, kind="ExternalOutput")

    n_it = n_blk // BPV
    n_pg = ND // PROJ_W
    it_per_pg = n_it // n_pg

    with tile.TileContext(nc) as tc, ExitStack() as ctx:
        const = ctx.enter_context(tc.tile_pool(name="const", bufs=1))
        gpool = ctx.enter_context(tc.tile_pool(name="g", bufs=4))
        spool = ctx.enter_context(tc.tile_pool(name="s", bufs=3))
        opool = ctx.enter_context(tc.tile_pool(name="o", bufs=2))
        psum = ctx.enter_context(tc.tile_pool(name="ps", bufs=3, space="PSUM"))
        psum2 = ctx.enter_context(tc.tile_pool(name="ps2", bufs=2, space="PSUM"))

        dr_t = const.tile([BLK, n_blk, nJ], mybir.dt.float16)
        nc.sync.dma_start(dr_t[:], drd[:])

        iota64 = const.tile([BLK, BPV, IBW], mybir.dt.float16)
        nc.gpsimd.iota(iota64[:], pattern=[[0, BPV], [1, IBW]], base=0,
                       channel_multiplier=0,
                       allow_small_or_imprecise_dtypes=True)
        iota16 = const.tile([BLK, BPV, n16, WIN], mybir.dt.float16)
        nc.gpsimd.iota(iota16[:], pattern=[[0, BPV], [0, n16], [1, WIN]],
                       base=0, channel_multiplier=0,
                       allow_small_or_imprecise_dtypes=True)
        iota8 = const.tile([BLK, BPV, n8, WIN], mybir.dt.float16)
        nc.gpsimd.iota(iota8[:], pattern=[[0, BPV], [0, n8], [1, WIN]],
                       base=0, channel_multiplier=0,
                       allow_small_or_imprecise_dtypes=True)

        wt_t = const.tile([F, F], mybir.dt.float16)
        deg_i = const.tile([1, ND], mybir.dt.int32)
        deg_f = const.tile([1, ND], mybir.dt.float32)
        rdeg1 = const.tile([1, ND], mybir.dt.float32)
        rdeg_b = const.tile([BLK, ND], mybir.dt.float32)
        yt_all = const.tile([BLK, ND], mybir.dt.float16)

        def emit_consts():
            nc.sync.dma_start(wt_t[:], wtd[:])
            nc.sync.dma_start(deg_i[:], degd[:])
            nc.vector.tensor_copy(deg_f[:], deg_i[:])
            nc.vector.reciprocal_approx_fast(rdeg1[:], deg_f[:])
            nc.gpsimd.partition_broadcast(rdeg_b[:], rdeg1[:])

        def emit_proj(pg):
            o_ps = psum2.tile([F, PROJ_W], mybir.dt.float32, tag="op")
            nc.tensor.matmul(
                o_ps[:], lhsT=wt_t[:],
                rhs=yt_all[:, pg * PROJ_W:(pg + 1) * PROJ_W],
                start=True, stop=True,
            )
            o_sb = opool.tile([F, PROJ_W], mybir.dt.float16, tag="ob")
            nc.vector.tensor_tensor(
                out=o_sb[:], in0=o_ps[:],
                in1=rdeg_b[:, pg * PROJ_W:(pg + 1) * PROJ_W],
                op=mybir.AluOpType.mult,
            )
            nc.scalar.dma_start(out[:, pg * PROJ_W:(pg + 1) * PROJ_W], o_sb[:])

        RW = rowb // 2
        for i in range(n_it):
            # one DMA for the iteration's BPV blocks: row r=j*128+p of the
            # BPV*128-row dram slab -> g2[p, j, :]
            g2 = gpool.tile([BLK, BPV, RW], mybir.dt.float16, tag="g")
            a = gin[i * BPV * BLK:(i + 1) * BPV * BLK, :]
            src = bass.AP(a.tensor, a.offset,
                          [[RW, BLK], [BLK * RW, BPV], [1, RW]])
            nc.sync.dma_start(g2[:], src)
            g_ts = [g2[:, j, :] for j in range(BPV)]
            if i == 1:
                emit_consts()

            s0 = spool.tile([BLK, BPV, IBW], mybir.dt.float16, tag="s0")
            nc.vector.tensor_tensor(
                out=s0[:],
                in0=iota64[:],
                in1=dr_t[:, i * BPV:(i + 1) * BPV, 0]
                .unsqueeze(2).broadcast_to([BLK, BPV, IBW]),
                op=mybir.AluOpType.is_equal,
            )
            s16 = spool.tile([BLK, BPV, n16, WIN], mybir.dt.float16, tag="s16")
            nc.vector.tensor_tensor(
                out=s16[:],
                in0=iota16[:],
                in1=dr_t[:, i * BPV:(i + 1) * BPV, 1:1 + n16]
                .unsqueeze(3).broadcast_to([BLK, BPV, n16, WIN]),
                op=mybir.AluOpType.is_equal,
            )
            s8 = spool.tile([BLK, BPV, n8, WIN], mybir.dt.float8e4, tag="s8")
            nc.vector.tensor_tensor(
                out=s8[:],
                in0=iota8[:],
                in1=dr_t[:, i * BPV:(i + 1) * BPV, 1 + n16:]
                .unsqueeze(3).broadcast_to([BLK, BPV, n8, WIN]),
                op=mybir.AluOpType.is_equal,
            )

            for j in range(BPV):
                b = i * BPV + j
                yt_ps = psum.tile([F, IBW], mybir.dt.float32, tag="yt")
                g_t = g_ts[j]
                nc.tensor.matmul(
                    yt_ps[:],
                    lhsT=g_t[:, offs[0] // 2:offs[1] // 2],
                    rhs=s0[:, j, :],
                    start=True, stop=False,
                    skip_group_check=True,
                )
                for q in range(1, nJ):
                    lo, w, is8 = cfg[q]
                    lhsT = g_t[:, offs[q] // 2:offs[q + 1] // 2]
                    if is8:
                        lhsT = lhsT.bitcast(mybir.dt.float8e4)
                    rhs = (s8[:, j, q - 1 - n16, :] if is8
                           else s16[:, j, q - 1, :])
                    nc.tensor.matmul(
                        yt_ps[:, lo:lo + w],
                        lhsT=lhsT, rhs=rhs,
                        start=False, stop=(q == nJ - 1),
                        skip_group_check=True,
                    )
                nc.scalar.activation(
                    yt_all[:, b * IBW:(b + 1) * IBW], yt_ps[:],
                    mybir.ActivationFunctionType.Copy,
                )

            if (i + 1) % it_per_pg == 0:
                emit_proj((i + 1) // it_per_pg - 1)

    nc.compile()
    return nc


_PROGRAM_CACHE = {}


def kernel(x, edge_index, W, B, profile_dir=None):
    x = np.ascontiguousarray(np.asarray(x), dtype=np.float32)
    edge_index = np.asarray(edge_index)
    W = np.asarray(W, dtype=np.float32)
    B = np.asarray(B, dtype=np.float32)
    N = x.shape[0]

    gin, dr, degc, K, n_blk, perm = _host_prep(x, edge_index)

    rowb = gin.shape[2] * 2  # bytes per block row (gin is fp16-typed bytes)
    ck = (N, n_blk, K, rowb)
    if ck not in _PROGRAM_CACHE:
        _PROGRAM_CACHE[ck] = _build_program(N, n_blk, K, rowb)
    nc = _PROGRAM_CACHE[ck]

    wt_np = np.ascontiguousarray(W.T.astype(np.float16))
    in_maps = [{
        "gin": gin[c],
        "dr": np.ascontiguousarray(dr[c]),
        "deg": np.ascontiguousarray(degc[c]),
        "wt": wt_np,
    } for c in range(N_CORES)]

    if profile_dir is not None:
        from trn_agent_boot.trn_boot import _ntff_profile_via_ctypes
        hook = _ntff_profile_via_ctypes("/opt/axon/libaxon_pjrt.so")
        os.makedirs(profile_dir, exist_ok=True)
        with hook(profile_dir, list(range(N_CORES))):
            res = run_bass_kernel_spmd(nc, in_maps, core_ids=list(range(N_CORES)))
    else:
        res = run_bass_kernel_spmd(nc, in_maps, core_ids=list(range(N_CORES)))

    rows = np.concatenate(
        [r["out"].T.astype(np.float32) for r in res.results], axis=0)
    out = np.empty_like(rows)
    out[perm] = rows  # un-permute balanced layout back to original dst ids

    if np.any(B):
        # B is zeros for this problem's inputs; exact fallback for generality.
        out = out + x @ B.T
    return out
